# revision 38
# baseline (speedup 1.0000x reference)
"""Radial k-space resampling (type-2 NUDFT -> density comp -> type-1 adjoint)
as a Bass/Tile kernel on 8 Trainium2 NeuronCores.

Fast path (used when the trajectory has the radial conjugate-symmetric
structure, which the reference's setup_inputs always produces): since x is
real, k(-k) = conj(k(k)), so each spoke's t and 384-t samples contribute
2*w*Re(k*E) to the adjoint. Each core processes only the 193 representative
samples per spoke (t=0..192) of its 16 spokes, with doubled weights for the
paired reps, and computes a real-only adjoint. The 16 unpaired t=0 samples
per core sit alone in the last m-chunk, where the imaginary adjoint is also
accumulated, keeping the result exact. Matmul operands and the adjoint-side
trig run in fp16 (1 PE cycle/row vs 4 for fp32); elementwise work is spread
across DVE/GpSimd/Act engines.

General fallback (arbitrary points): full complex pipeline over all 49152
points, 6144 per core.

Host dispatch: the jitted executables, staged device inputs, and (unread)
output-operand buffers are cached across kernel() calls -- the axon tunnel
has ~70ms/RPC latency, so warm calls cost about one round trip. The 8
partial images are summed on-device (all-reduce) and only the final image
is fetched.

Self-contained: hardcodes all shapes from the problem spec.
"""

import sys

sys.path.insert(0, "/opt/trn_rl_repo")

import math
from contextlib import ExitStack

import numpy as np

import concourse.tile as tile
from concourse import bacc, mybir
from concourse import bass2jax

F32 = mybir.dt.float32
F16 = mybir.dt.float16
AF = mybir.ActivationFunctionType
ALU = mybir.AluOpType

H = 192          # image height (= width)
W = 192
CB = 4           # C*B images
M_TOT = 49152
N_CORES = 8
MC = 128                       # m-chunk (partition tile)
PAIRS = 2                      # image pairs: (0,1) and (2,3)
PW = 2 * W                     # 384 = pair width in wcb columns

# general path
MG_CORE = M_TOT // N_CORES     # 6144
NG_CHUNK = MG_CORE // MC       # 48
SGG = 8                        # chunks per sub-group
NG_SG = NG_CHUNK // SGG        # 6

# symmetric path
SPOKES = 128
TSAMP = 384
RT = 193                       # representative samples per spoke (t=0..192)
NSPK = SPOKES // N_CORES       # 16 spokes per core
MS_REAL = NSPK * RT            # 3088
MS_CORE = 3200                 # padded to 25 chunks
NS_CHUNK = MS_CORE // MC       # 25
SGS = 5
NS_SG = NS_CHUNK // SGS        # 5
# sub-group segments (start_chunk, n_chunks): a 1-chunk prologue lets the
# first forward/accum work start ~4x earlier (pipeline fill)
SEGS = ((0, 1), (1, 4), (5, 5), (10, 5), (15, 5), (20, 4), (24, 1))

TWO_PI = 2.0 * math.pi
INV2PI = 1.0 / TWO_PI
MAGIC = 12582912.0                 # 2^23 + 2^22: fp32 round-to-nearest shifter

_CACHE = {}


# ---------------------------------------------------------------------------
# symmetric-path program
# ---------------------------------------------------------------------------

def _build_sym_program():
    nc = bacc.Bacc("TRN2", target_bir_lowering=False, debug=False,
                   num_devices=1)

    xs_d = nc.dram_tensor("xs", [H, CB * W], F16, kind="ExternalInput").ap()
    kxr_d = nc.dram_tensor("kxr", [1, MS_CORE], F32, kind="ExternalInput").ap()
    kxc_d = nc.dram_tensor("kxc", [MC, NS_CHUNK], F32, kind="ExternalInput").ap()
    kyc_d = nc.dram_tensor("kyc", [MC, NS_CHUNK], F32, kind="ExternalInput").ap()
    wd_d = nc.dram_tensor("wd", [MC, 2 * NS_CHUNK], F32, kind="ExternalInput").ap()
    wnd_d = nc.dram_tensor("wnd", [MC, 2 * NS_CHUNK], F32, kind="ExternalInput").ap()
    grid_d = nc.dram_tensor("grid", [1, H], F32, kind="ExternalInput").ap()
    nxc_d = nc.dram_tensor("nxc", [MC, 2], F32, kind="ExternalInput").ap()
    out_d = nc.dram_tensor("out", [MC, 2 * 6 * H], F32, kind="ExternalOutput").ap()

    with tile.TileContext(nc) as tc:
        with ExitStack() as ctx:
            _sym_body(ctx, tc, xs_d, kxr_d, kxc_d, kyc_d, wd_d, wnd_d,
                      grid_d, nxc_d, out_d)

    nc.compile()
    return nc


def _sym_body(ctx, tc, xs_d, kxr_d, kxc_d, kyc_d, wd_d, wnd_d,
              grid_d, nxc_d, out_d):
    nc = tc.nc

    statics = ctx.enter_context(tc.tile_pool(name="statics", bufs=1))
    trig = ctx.enter_context(tc.tile_pool(name="trig", bufs=2))
    scratch = ctx.enter_context(tc.tile_pool(name="scratch", bufs=1))
    small = ctx.enter_context(tc.tile_pool(name="small", bufs=4))
    gpool = ctx.enter_context(tc.tile_pool(name="gpool", bufs=2))
    psum_tmp = ctx.enter_context(tc.tile_pool(name="ptmp", bufs=1, space="PSUM"))
    psum_adj = ctx.enter_context(tc.tile_pool(name="padj", bufs=1, space="PSUM"))

    # ---- static loads (trig-critical tensors first) ----
    xs_a = statics.tile([128, CB * W], F16)
    xs_b = statics.tile([64, CB * W], F16)

    kxc = statics.tile([MC, NS_CHUNK], F32)
    kyc = statics.tile([MC, NS_CHUNK], F32)
    wd = statics.tile([MC, 2 * NS_CHUNK], F32)
    wnd = statics.tile([MC, 2 * NS_CHUNK], F32)
    nxc = statics.tile([MC, 2], F32)
    kxr_all = statics.tile([1, MS_CORE], F32)
    nc.sync.dma_start(kxr_all[:], kxr_d[:, :])
    nc.sync.dma_start(nxc[:], nxc_d[:, :])
    grid_row = statics.tile([1, H], F32)
    nc.sync.dma_start(grid_row[:], grid_d[:, :])
    nc.sync.dma_start(kxc[:], kxc_d[:, :])
    nc.sync.dma_start(kyc[:], kyc_d[:, :])
    grid_bc = statics.tile([MC, H], F32)
    nc.gpsimd.partition_broadcast(grid_bc[:], grid_row[:])

    nc.sync.dma_start(wd[:], wd_d[:, :])
    nc.sync.dma_start(wnd[:], wnd_d[:, :])
    nc.sync.dma_start(xs_a[:], xs_d[0:128, :])
    nc.sync.dma_start(xs_b[:], xs_d[128:192, :])

    b_hpi = statics.tile([MC, 1], F32)
    nc.vector.memset(b_hpi[:], math.pi / 2.0)

    # acc: [w'(128), re(6x192) | im(6x192)]; no memset -- the first flush
    # of every region is a copy
    acc = statics.tile([MC, 2 * 6 * H], F32)

    def _reduce_arg(pool, ang_t, p, fd, tag, affine_eng):
        # arg = ang - 2*pi*round(ang/2*pi), clamped to [-pi, pi], on the
        # [0:p, 0:fd] slice. Affine rounding on Act or Pool (by caller),
        # the tensor-tensor step on DVE or Pool, clamp on Pool.
        full = ang_t.shape[1]
        ang = ang_t[0:p, 0:fd]
        r_t = pool.tile([ang_t.shape[0], full], F32, tag=tag[:2] + "_r")
        r = r_t[0:p, 0:fd]
        if affine_eng == "act":
            nc.scalar.activation(r, ang, AF.Copy, scale=INV2PI, bias=MAGIC)
            nc.scalar.activation(r, r, AF.Copy, bias=-MAGIC)
        else:
            nc.gpsimd.tensor_scalar(r, ang, INV2PI, MAGIC,
                                    op0=ALU.mult, op1=ALU.add)
            nc.gpsimd.tensor_scalar_sub(r, r, MAGIC)
        arg_t = pool.tile([ang_t.shape[0], full], F32, tag=tag[:2] + "_a",
                          bufs=2)
        arg = arg_t[0:p, 0:fd]
        if affine_eng == "act":
            nc.gpsimd.tensor_scalar_mul(r, r, -TWO_PI)
            nc.gpsimd.tensor_tensor(arg, r, ang, op=ALU.add)
        else:
            nc.vector.scalar_tensor_tensor(arg, r, -TWO_PI, ang,
                                           op0=ALU.mult, op1=ALU.add)
        nc.gpsimd.tensor_scalar(arg, arg, math.pi, -math.pi,
                                op0=ALU.min, op1=ALU.max)
        return arg_t

    def _emit_kx_bc(si2, st2, ln2):
        kxb = scratch.tile([MC, SGS * MC], F32, tag="kxbc", bufs=2,
                           name=f"kxbc_{si2}")
        nc.gpsimd.partition_broadcast(
            kxb[:, 0:ln2 * MC],
            kxr_all[0:1, st2 * MC:(st2 + ln2) * MC])
        return kxb

    kx_next = _emit_kx_bc(0, *SEGS[0])
    for si, (st, ln) in enumerate(SEGS):
        m0 = st * MC
        last_seg = si == len(SEGS) - 1

        # ===== trig for this segment (kx broadcast emitted a segment
        # ahead so it clears the Pool queue early) =====
        kx_bc = kx_next
        if not last_seg:
            kx_next = _emit_kx_bc(si + 1, *SEGS[si + 1])

        # forward lhsT trig [h, m] in f16 (h split 128+64)
        cos_hm_a = trig.tile([128, SGS * MC], F16, tag="chma")
        sin_hm_a = trig.tile([128, SGS * MC], F16, tag="shma")
        cos_hm_b = trig.tile([64, SGS * MC], F16, tag="chmb")
        sin_hm_b = trig.tile([64, SGS * MC], F16, tag="shmb")
        cs_hm_a = trig.tile([128, SGS * MC], F16, tag="cshma")
        cs_hm_b = trig.tile([64, SGS * MC], F16, tag="cshmb")
        for (ht, p, ctile, stile, cstile) in (
                (0, 128, cos_hm_a, sin_hm_a, cs_hm_a),
                (1, 64, cos_hm_b, sin_hm_b, cs_hm_b)):
            fm = ln * MC
            ang = scratch.tile([p, SGS * MC], F32, tag="ang_hm", bufs=2)
            nc.vector.tensor_scalar_mul(ang[0:p, 0:fm], kx_bc[0:p, 0:fm],
                                        nxc[0:p, ht:ht + 1])
            argS = _reduce_arg(scratch, ang, p, fm, "hm_s", "pool")
            nc.scalar.activation(stile[0:p, 0:fm], argS[0:p, 0:fm],
                                 AF.Sin, bias=0.0)
            ab = scratch.tile([p, SGS * MC], F32, tag="hm_b", bufs=2)
            nc.scalar.activation(ab[0:p, 0:fm], argS[0:p, 0:fm], AF.Abs)
            nc.scalar.activation(ctile[0:p, 0:fm], ab[0:p, 0:fm], AF.Sin,
                                 bias=b_hpi[0:p], scale=-1.0)
            nc.gpsimd.tensor_tensor(cstile[0:p, 0:fm], ctile[0:p, 0:fm],
                                    stile[0:p, 0:fm], op=ALU.add)

        # adjoint rhs trig [m, h] in f16; k-dot trig [m, w] in f32 + f16
        cosH = trig.tile([MC, SGS * H], F16, tag="cH")
        nsinH = trig.tile([MC, SGS * H], F16, tag="nsH")
        sinH = (trig.tile([MC, SGS * H], F16, tag="sH", name=f"sinH_{si}")
                if last_seg else None)
        cosWh = trig.tile([MC, SGS * H], F16, tag="cWh")
        sinWh = trig.tile([MC, SGS * H], F16, tag="sWh")
        csW = trig.tile([MC, SGS * H], F16, tag="csW")

        fh = ln * H
        for side, kcol in (("mh", kxc), ("mw", kyc)):
            ang8 = scratch.tile([MC, SGS * H], F32, tag="ang8" + side,
                                 bufs=2)
            for c in range(ln):
                ch = st + c
                nc.gpsimd.tensor_scalar_mul(ang8[:, c * H:(c + 1) * H],
                                            grid_bc[:], kcol[:, ch:ch + 1])
            argS8 = _reduce_arg(scratch, ang8, MC, fh, side + "_s", "act")
            ab8 = scratch.tile([MC, SGS * H], F32, tag=side + "_b", bufs=2)
            nc.scalar.activation(ab8[:, 0:fh], argS8[:, 0:fh], AF.Abs)
            if side == "mh":
                nc.scalar.activation(nsinH[:, 0:fh], argS8[:, 0:fh],
                                     AF.Sin, bias=0.0, scale=-1.0)
                nc.scalar.activation(cosH[:, 0:fh], ab8[:, 0:fh], AF.Sin,
                                     bias=b_hpi[:], scale=-1.0)
                if sinH is not None:
                    nc.scalar.activation(sinH[:, 0:fh], argS8[:, 0:fh],
                                         AF.Sin, bias=0.0)
            else:
                nc.scalar.activation(sinWh[:, 0:fh], argS8[:, 0:fh],
                                     AF.Sin, bias=0.0)
                nc.scalar.activation(cosWh[:, 0:fh], ab8[:, 0:fh], AF.Sin,
                                     bias=b_hpi[:], scale=-1.0)
                nc.gpsimd.tensor_tensor(csW[:, 0:fh], cosWh[:, 0:fh],
                                        sinWh[:, 0:fh], op=ALU.add)

        # ===== compute: per pair =====
        for pair in range(PAIRS):
            a1t = small.tile([MC, 2 * SGS], F32, tag="a1t")
            a2t = small.tile([MC, 2 * SGS], F32, tag="a2t")
            a3t = small.tile([MC, 2 * SGS], F32, tag="a3t")

            for c in range(ln):
                ms = c * MC

                tmp_re = psum_tmp.tile([MC, PW], F32, tag="tre",
                                       name=f"tre_{si}_{pair}_{c}")
                tmp_imn = psum_tmp.tile([MC, PW], F32, tag="tim",
                                        name=f"tim_{si}_{pair}_{c}")
                tmp_sum = psum_tmp.tile([MC, PW], F32, tag="tsum",
                                        name=f"tsum_{si}_{pair}_{c}")
                xa = xs_a[:, pair * PW:(pair + 1) * PW]
                xb = xs_b[:, pair * PW:(pair + 1) * PW]
                nc.tensor.matmul(tmp_re[:], cos_hm_a[:, ms:ms + MC], xa,
                                 start=True, stop=False)
                nc.tensor.matmul(tmp_re[:], cos_hm_b[:, ms:ms + MC], xb,
                                 start=False, stop=True)
                nc.tensor.matmul(tmp_imn[:], sin_hm_a[:, ms:ms + MC], xa,
                                 start=True, stop=False)
                nc.tensor.matmul(tmp_imn[:], sin_hm_b[:, ms:ms + MC], xb,
                                 start=False, stop=True)
                nc.tensor.matmul(tmp_sum[:], cs_hm_a[:, ms:ms + MC], xa,
                                 start=True, stop=False)
                nc.tensor.matmul(tmp_sum[:], cs_hm_b[:, ms:ms + MC], xb,
                                 start=False, stop=True)

                hs = slice(c * H, (c + 1) * H)
                # k-dot accumulations (Karatsuba: 3 per image). Fused
                # mult+row-reduce is DVE-only (stt can't run on Pool, and
                # only DVE/Act may read PSUM).
                for im in range(2):
                    ws = slice(im * W, (im + 1) * W)
                    col = im * ln + c
                    junkv = scratch.tile([MC, W], F32, tag="junkv", bufs=2)
                    nc.vector.scalar_tensor_tensor(
                        junkv[:], tmp_re[:, ws], 1.0, cosWh[:, hs],
                        op0=ALU.mult, op1=ALU.mult,
                        accum_out=a1t[:, col:col + 1])
                    nc.vector.scalar_tensor_tensor(
                        junkv[:], tmp_imn[:, ws], 1.0, sinWh[:, hs],
                        op0=ALU.mult, op1=ALU.mult,
                        accum_out=a2t[:, col:col + 1])
                    nc.vector.scalar_tensor_tensor(
                        junkv[:], tmp_sum[:, ws], 1.0, csW[:, hs],
                        op0=ALU.mult, op1=ALU.mult,
                        accum_out=a3t[:, col:col + 1])

            # batched k and weighting: kre = a1-a2, kimn = a3+a4,
            # kw_re = w*kre, kw_im = -w*kimn
            nb = 2 * ln
            ds = slice(2 * st, 2 * st + nb)
            kre_t = small.tile([MC, 2 * SGS], F32, tag="kret")
            kpp_t = small.tile([MC, 2 * SGS], F32, tag="kppt")
            kimn_t = small.tile([MC, 2 * SGS], F32, tag="kimt")
            kwre_t = small.tile([MC, 2 * SGS], F32, tag="kwret")
            kwim_t = small.tile([MC, 2 * SGS], F32, tag="kwimt")
            kwsum_t = small.tile([MC, 2 * SGS], F32, tag="kwst")
            kwdif_t = small.tile([MC, 2 * SGS], F32, tag="kwdt")
            nc.gpsimd.tensor_tensor(kre_t[:, 0:nb], a1t[:, 0:nb],
                                    a2t[:, 0:nb], op=ALU.subtract)
            nc.gpsimd.tensor_tensor(kpp_t[:, 0:nb], a1t[:, 0:nb],
                                    a2t[:, 0:nb], op=ALU.add)
            nc.gpsimd.tensor_tensor(kimn_t[:, 0:nb], a3t[:, 0:nb],
                                    kpp_t[:, 0:nb], op=ALU.subtract)
            nc.gpsimd.tensor_tensor(kwre_t[:, 0:nb], kre_t[:, 0:nb],
                                    wd[:, ds], op=ALU.mult)
            nc.gpsimd.tensor_tensor(kwim_t[:, 0:nb], kimn_t[:, 0:nb],
                                    wnd[:, ds], op=ALU.mult)
            nc.gpsimd.tensor_tensor(kwsum_t[:, 0:nb], kwre_t[:, 0:nb],
                                    kwim_t[:, 0:nb], op=ALU.add)
            nc.gpsimd.tensor_tensor(kwdif_t[:, 0:nb], kwre_t[:, 0:nb],
                                    kwim_t[:, 0:nb], op=ALU.subtract)

            adj_re = [psum_adj.tile([128, H], F32, tag=f"adjre{t}",
                                    name=f"adjre_{si}_{pair}_{t}")
                      for t in range(3)]

            for c in range(ln):
                hs = slice(c * H, (c + 1) * H)
                g_re = gpool.tile([MC, PW], F16, tag="gre")
                g_im = gpool.tile([MC, PW], F16, tag="gim")
                drain = last_seg and pair == PAIRS - 1
                for im in range(2):
                    ws = slice(im * W, (im + 1) * W)
                    col = im * ln + c
                    ta = small.tile([MC, W], F16, tag="ta")
                    tb = small.tile([MC, W], F16, tag="tb")
                    tc_ = small.tile([MC, W], F16, tag="tc")
                    if drain:
                        # tail of the pipeline: DVE is idle here, Pool/Act
                        # are the drain bottleneck
                        nc.vector.tensor_scalar_mul(ta[:], csW[:, hs],
                                                    kwre_t[:, col:col + 1])
                        nc.vector.tensor_scalar_mul(tb[:], sinWh[:, hs],
                                                    kwsum_t[:, col:col + 1])
                        nc.vector.tensor_scalar_mul(tc_[:], cosWh[:, hs],
                                                    kwdif_t[:, col:col + 1])
                        nc.vector.tensor_tensor(g_re[:, ws], ta[:], tb[:],
                                                op=ALU.subtract)
                        nc.vector.tensor_tensor(g_im[:, ws], ta[:], tc_[:],
                                                op=ALU.subtract)
                        continue
                    nc.gpsimd.tensor_scalar_mul(ta[:], csW[:, hs],
                                                kwre_t[:, col:col + 1])
                    if im == 0:
                        nc.gpsimd.tensor_scalar_mul(tb[:], sinWh[:, hs],
                                                    kwsum_t[:, col:col + 1])
                    else:
                        nc.scalar.activation(tb[:], sinWh[:, hs], AF.Copy,
                                             scale=kwsum_t[:, col:col + 1])
                    if im == 0:
                        nc.gpsimd.tensor_scalar_mul(tc_[:], cosWh[:, hs],
                                                    kwdif_t[:, col:col + 1])
                    else:
                        nc.scalar.activation(tc_[:], cosWh[:, hs], AF.Copy,
                                             scale=kwdif_t[:, col:col + 1])
                    nc.gpsimd.tensor_tensor(g_re[:, ws], ta[:], tb[:],
                                            op=ALU.subtract)
                    nc.gpsimd.tensor_tensor(g_im[:, ws], ta[:], tc_[:],
                                            op=ALU.subtract)

                first = c == 0
                last = c == ln - 1
                for t in range(3):
                    gl = slice(t * 128, (t + 1) * 128)
                    nc.tensor.matmul(adj_re[t][:], g_re[:, gl],
                                     cosH[:, hs], start=first, stop=False)
                    nc.tensor.matmul(adj_re[t][:], g_im[:, gl],
                                     nsinH[:, hs], start=False, stop=last)
            for t in range(3):
                r0 = (pair * 3 + t) * H
                if si == 0:
                    nc.vector.tensor_copy(acc[:, r0:r0 + H], adj_re[t][:])
                else:
                    nc.vector.tensor_tensor(acc[:, r0:r0 + H],
                                            acc[:, r0:r0 + H],
                                            adj_re[t][:], op=ALU.add)

            if last_seg:
                # unpaired t=0 points (last chunk only): imaginary adjoint,
                # reusing the freshly-flushed adj_re PSUM banks
                hs = slice((ln - 1) * H, ln * H)
                for t in range(3):
                    gl = slice(t * 128, (t + 1) * 128)
                    adj_im = psum_adj.tile(
                        [128, H], F32, tag=f"adjre{t}",
                        name=f"adjim_{pair}_{t}")
                    nc.tensor.matmul(adj_im[:], g_re[:, gl],
                                     sinH[:, hs], start=True, stop=False)
                    nc.tensor.matmul(adj_im[:], g_im[:, gl],
                                     cosH[:, hs], start=False, stop=True)
                    r0 = 6 * H + (pair * 3 + t) * H
                    nc.vector.tensor_copy(acc[:, r0:r0 + H], adj_im[:])

    nc.sync.dma_start(out_d[:, 0:3 * H], acc[:, 0:3 * H])
    nc.sync.dma_start(out_d[:, 3 * H:6 * H], acc[:, 3 * H:6 * H])
    nc.sync.dma_start(out_d[:, 6 * H:9 * H], acc[:, 6 * H:9 * H])
    nc.sync.dma_start(out_d[:, 9 * H:], acc[:, 9 * H:])


def _prep_sym_inputs(x, points, weights):
    """Host-side prep for the symmetric path -> per-core in_maps."""
    x = np.asarray(x, dtype=np.float32)
    xs = np.transpose(x, (1, 3, 0, 2)).reshape(H, CB * W)
    xs16 = xs.astype(np.float16)

    pr = np.remainder(np.asarray(points, np.float32) + np.pi, TWO_PI)
    pr = (pr.astype(np.float64) - np.pi).astype(np.float32)
    pr3 = pr.reshape(SPOKES, TSAMP, 2)
    w2 = (np.asarray(weights, np.float64) / float(H * W)).reshape(SPOKES, TSAMP)

    wr = w2[:, :RT].copy()
    wr[:, 1:TSAMP // 2] *= 2.0          # paired reps t=1..191
    wr = wr.astype(np.float32)

    grid = (np.arange(H, dtype=np.float32) - H // 2).reshape(1, H)
    nxc = np.zeros((MC, 2), dtype=np.float32)
    nxc[:, 0] = grid[0, 0:128]
    nxc[0:64, 1] = grid[0, 128:192]

    in_maps = []
    for c in range(N_CORES):
        sp = slice(c * NSPK, (c + 1) * NSPK)
        kx_s = pr3[sp, :RT, 0]
        ky_s = pr3[sp, :RT, 1]
        w_s = wr[sp]
        npad = MS_CORE - MS_REAL
        # paired+DC reps (t=1..192) first, unpaired t=0 last, zero-pad tail
        kx = np.concatenate([kx_s[:, 1:].ravel(), kx_s[:, 0],
                             np.zeros(npad, np.float32)])
        ky = np.concatenate([ky_s[:, 1:].ravel(), ky_s[:, 0],
                             np.zeros(npad, np.float32)])
        ws = np.concatenate([w_s[:, 1:].ravel(), w_s[:, 0],
                             np.zeros(npad, np.float32)])
        wcol = ws.reshape(NS_CHUNK, MC).T          # [128, 25]
        wdup = np.zeros((MC, 2 * NS_CHUNK), np.float32)
        for (st, ln) in SEGS:
            for im in range(2):
                for cc in range(ln):
                    wdup[:, 2 * st + im * ln + cc] = wcol[:, st + cc]
        in_maps.append({
            "xs": xs16,
            "kxr": kx.reshape(1, MS_CORE),
            "kxc": kx.reshape(NS_CHUNK, MC).T.copy(),
            "kyc": ky.reshape(NS_CHUNK, MC).T.copy(),
            "wd": wdup,
            "wnd": -wdup,
            "grid": grid,
            "nxc": nxc,
        })
    return in_maps


def _sym_applicable(points, weights):
    pts3 = np.asarray(points).reshape(SPOKES, TSAMP, 2)
    w2 = np.asarray(weights).reshape(SPOKES, TSAMP)
    ht = TSAMP // 2
    return (np.array_equal(pts3[:, 1:ht, :], -pts3[:, ht + 1:, :][:, ::-1, :])
            and np.array_equal(w2[:, 1:ht], w2[:, ht + 1:][:, ::-1]))


# ---------------------------------------------------------------------------
# general-path program (arbitrary points; full complex pipeline)
# ---------------------------------------------------------------------------

def _build_program():
    nc = bacc.Bacc("TRN2", target_bir_lowering=False, debug=False,
                   num_devices=1)

    xs_d = nc.dram_tensor("xs", [H, CB * W], F32, kind="ExternalInput").ap()
    kxr_d = nc.dram_tensor("kxr", [1, MG_CORE], F32, kind="ExternalInput").ap()
    kxc_d = nc.dram_tensor("kxc", [MC, NG_CHUNK], F32, kind="ExternalInput").ap()
    kyc_d = nc.dram_tensor("kyc", [MC, NG_CHUNK], F32, kind="ExternalInput").ap()
    wc_d = nc.dram_tensor("wc", [MC, NG_CHUNK], F32, kind="ExternalInput").ap()
    wnc_d = nc.dram_tensor("wnc", [MC, NG_CHUNK], F32, kind="ExternalInput").ap()
    grid_d = nc.dram_tensor("grid", [1, H], F32, kind="ExternalInput").ap()
    nxc_d = nc.dram_tensor("nxc", [MC, 2], F32, kind="ExternalInput").ap()
    out_d = nc.dram_tensor("out", [MC, 6 * 2 * H], F32, kind="ExternalOutput").ap()

    with tile.TileContext(nc) as tc:
        with ExitStack() as ctx:
            _gen_body(ctx, tc, xs_d, kxr_d, kxc_d, kyc_d, wc_d, wnc_d,
                      grid_d, nxc_d, out_d)

    nc.compile()
    return nc


def _gen_body(ctx, tc, xs_d, kxr_d, kxc_d, kyc_d, wc_d, wnc_d,
              grid_d, nxc_d, out_d):
    nc = tc.nc

    statics = ctx.enter_context(tc.tile_pool(name="statics", bufs=1))
    trig = ctx.enter_context(tc.tile_pool(name="trig", bufs=2))
    scratch = ctx.enter_context(tc.tile_pool(name="scratch", bufs=1))
    small = ctx.enter_context(tc.tile_pool(name="small", bufs=4))
    gpool = ctx.enter_context(tc.tile_pool(name="gpool", bufs=2))
    psum_tmp = ctx.enter_context(tc.tile_pool(name="ptmp", bufs=1, space="PSUM"))
    psum_adj = ctx.enter_context(tc.tile_pool(name="padj", bufs=1, space="PSUM"))

    xs_a = statics.tile([128, CB * W], F32)
    xs_b = statics.tile([64, CB * W], F32)
    nc.sync.dma_start(xs_a[:], xs_d[0:128, :])
    nc.sync.dma_start(xs_b[:], xs_d[128:192, :])

    kxc = statics.tile([MC, NG_CHUNK], F32)
    kyc = statics.tile([MC, NG_CHUNK], F32)
    wc = statics.tile([MC, NG_CHUNK], F32)
    wnc = statics.tile([MC, NG_CHUNK], F32)
    nxc = statics.tile([MC, 2], F32)
    nc.sync.dma_start(kxc[:], kxc_d[:, :])
    nc.sync.dma_start(kyc[:], kyc_d[:, :])
    nc.sync.dma_start(wc[:], wc_d[:, :])
    nc.sync.dma_start(wnc[:], wnc_d[:, :])
    nc.sync.dma_start(nxc[:], nxc_d[:, :])

    grid_row = statics.tile([1, H], F32)
    nc.sync.dma_start(grid_row[:], grid_d[:, :])
    grid_bc = statics.tile([MC, H], F32)
    nc.gpsimd.partition_broadcast(grid_bc[:], grid_row[:])

    nc.sync.dma_start(xs_a[:], xs_d[0:128, :])
    nc.sync.dma_start(xs_b[:], xs_d[128:192, :])

    b_hpi = statics.tile([MC, 1], F32)
    nc.vector.memset(b_hpi[:], math.pi / 2.0)

    def _reduce_arg(pool, ang, tag):
        p, fd = ang.shape[0], ang.shape[1]
        r = pool.tile([p, fd], F32, tag=tag[:2] + "_r")
        nc.vector.tensor_scalar(r[:], ang[:], INV2PI, MAGIC,
                                op0=ALU.mult, op1=ALU.add)
        nc.vector.tensor_scalar_sub(r[:], r[:], MAGIC)
        arg = pool.tile([p, fd], F32, tag=tag[:2] + "_a", bufs=2)
        nc.vector.scalar_tensor_tensor(arg[:], r[:], -TWO_PI, ang[:],
                                       op0=ALU.mult, op1=ALU.add)
        nc.vector.tensor_scalar(arg[:], arg[:], math.pi, -math.pi,
                                op0=ALU.min, op1=ALU.max)
        return arg

    def _emit_trig(pool, arg, stile, ctile, ntile, tag):
        p = arg.shape[0]
        nc.scalar.activation(stile[:], arg[:], AF.Sin, bias=0.0)
        if ntile is not None:
            nc.scalar.activation(ntile[:], arg[:], AF.Sin, bias=0.0, scale=-1.0)
        ab = pool.tile([p, arg.shape[1]], F32, tag=tag[:2] + "_b", bufs=2)
        nc.scalar.activation(ab[:], arg[:], AF.Abs)
        nc.scalar.activation(ctile[:], ab[:], AF.Sin, bias=b_hpi[0:p], scale=-1.0)

    acc = statics.tile([MC, 6 * 2 * H], F32)
    nc.vector.memset(acc[:], 0.0)

    for sg in range(NG_SG):
        m0 = sg * SGG * MC

        kx_row = small.tile([1, SGG * MC], F32, tag="kxrow")
        nc.sync.dma_start(kx_row[:], kxr_d[0:1, m0:m0 + SGG * MC])
        kx_bc = scratch.tile([MC, SGG * MC], F32, tag="kxbc", bufs=2)
        nc.gpsimd.partition_broadcast(kx_bc[:], kx_row[:])

        cos_hm_a = trig.tile([128, SGG * MC], F32, tag="chma")
        sin_hm_a = trig.tile([128, SGG * MC], F32, tag="shma")
        cos_hm_b = trig.tile([64, SGG * MC], F32, tag="chmb")
        sin_hm_b = trig.tile([64, SGG * MC], F32, tag="shmb")
        for (ht, p, ctile, stile) in ((0, 128, cos_hm_a, sin_hm_a),
                                      (1, 64, cos_hm_b, sin_hm_b)):
            ang = scratch.tile([p, SGG * MC], F32, tag="ang_hm")
            nc.vector.tensor_scalar_mul(ang[:], kx_bc[0:p, :], nxc[0:p, ht:ht + 1])
            argS = _reduce_arg(scratch, ang, "hm_s")
            _emit_trig(scratch, argS, stile, ctile, None, "hm_s")

        cos_mh = trig.tile([MC, SGG * H], F32, tag="cmh")
        sin_mh = trig.tile([MC, SGG * H], F32, tag="smh")
        nsin_mh = trig.tile([MC, SGG * H], F32, tag="nsmh")
        cos_mw = trig.tile([MC, SGG * W], F32, tag="cmw")
        sin_mw = trig.tile([MC, SGG * W], F32, tag="smw")

        for (kcol, ctile, stile, ntile) in (
                (kxc, cos_mh, sin_mh, nsin_mh),
                (kyc, cos_mw, sin_mw, None)):
            ang8 = scratch.tile([MC, SGG * H], F32, tag="ang8")
            for c in range(SGG):
                ch = sg * SGG + c
                nc.vector.tensor_scalar_mul(ang8[:, c * H:(c + 1) * H],
                                            grid_bc[:], kcol[:, ch:ch + 1])
            argS8 = _reduce_arg(scratch, ang8, "m8_s")
            _emit_trig(scratch, argS8, stile, ctile, ntile, "m8_s")

        for pair in range(PAIRS):
            adj_re = [psum_adj.tile([128, H], F32, tag=f"adjre{t}",
                                    name=f"adjre_{sg}_{pair}_{t}")
                      for t in range(3)]
            adj_im = [psum_adj.tile([128, H], F32, tag=f"adjim{t}",
                                    name=f"adjim_{sg}_{pair}_{t}")
                      for t in range(3)]
            for c in range(SGG):
                ch = sg * SGG + c
                ms = c * MC
                hs = slice(c * H, (c + 1) * H)

                tmp_re = psum_tmp.tile([MC, PW], F32, tag="tre")
                tmp_imn = psum_tmp.tile([MC, PW], F32, tag="tim")
                xa = xs_a[:, pair * PW:(pair + 1) * PW]
                xb = xs_b[:, pair * PW:(pair + 1) * PW]
                nc.tensor.matmul(tmp_re[:], cos_hm_a[:, ms:ms + MC], xa,
                                 start=True, stop=False)
                nc.tensor.matmul(tmp_re[:], cos_hm_b[:, ms:ms + MC], xb,
                                 start=False, stop=True)
                nc.tensor.matmul(tmp_imn[:], sin_hm_a[:, ms:ms + MC], xa,
                                 start=True, stop=False)
                nc.tensor.matmul(tmp_imn[:], sin_hm_b[:, ms:ms + MC], xb,
                                 start=False, stop=True)

                g_re = gpool.tile([MC, PW], F32, tag="gre")
                g_im = gpool.tile([MC, PW], F32, tag="gim")
                junk = scratch.tile([MC, W], F32, tag="junk", bufs=2)
                for im in range(2):
                    ws = slice(im * W, (im + 1) * W)
                    a1 = small.tile([MC, 1], F32, tag="a1")
                    a2 = small.tile([MC, 1], F32, tag="a2")
                    a3 = small.tile([MC, 1], F32, tag="a3")
                    a4 = small.tile([MC, 1], F32, tag="a4")
                    kre = small.tile([MC, 1], F32, tag="kre")
                    kimn = small.tile([MC, 1], F32, tag="kimn")
                    nc.vector.scalar_tensor_tensor(
                        junk[:], tmp_re[:, ws], 1.0, cos_mw[:, hs],
                        op0=ALU.mult, op1=ALU.mult, accum_out=a1[:])
                    nc.vector.scalar_tensor_tensor(
                        junk[:], tmp_imn[:, ws], 1.0, sin_mw[:, hs],
                        op0=ALU.mult, op1=ALU.mult, accum_out=a2[:])
                    nc.vector.scalar_tensor_tensor(
                        junk[:], tmp_re[:, ws], 1.0, sin_mw[:, hs],
                        op0=ALU.mult, op1=ALU.mult, accum_out=a3[:])
                    nc.vector.scalar_tensor_tensor(
                        junk[:], tmp_imn[:, ws], 1.0, cos_mw[:, hs],
                        op0=ALU.mult, op1=ALU.mult, accum_out=a4[:])
                    nc.vector.tensor_tensor(kre[:], a1[:], a2[:],
                                            op=ALU.subtract)
                    nc.vector.tensor_tensor(kimn[:], a3[:], a4[:],
                                            op=ALU.add)

                    kw_re = small.tile([MC, 1], F32, tag="kwre")
                    kw_im = small.tile([MC, 1], F32, tag="kwim")
                    nc.vector.tensor_scalar_mul(kw_re[:], kre[:], wc[:, ch:ch + 1])
                    nc.vector.tensor_scalar_mul(kw_im[:], kimn[:], wnc[:, ch:ch + 1])

                    t1 = small.tile([MC, W], F32, tag="t1")
                    t2 = small.tile([MC, W], F32, tag="t2")
                    nc.vector.tensor_scalar_mul(t1[:], sin_mw[:, hs], kw_im[:])
                    nc.vector.scalar_tensor_tensor(
                        g_re[:, im * W:(im + 1) * W], cos_mw[:, hs], kw_re[:],
                        t1[:], op0=ALU.mult, op1=ALU.subtract)
                    nc.vector.tensor_scalar_mul(t2[:], cos_mw[:, hs], kw_im[:])
                    nc.vector.scalar_tensor_tensor(
                        g_im[:, im * W:(im + 1) * W], sin_mw[:, hs], kw_re[:],
                        t2[:], op0=ALU.mult, op1=ALU.add)

                first = c == 0
                last = c == SGG - 1
                for t in range(3):
                    gl = slice(t * 128, (t + 1) * 128)
                    nc.tensor.matmul(adj_re[t][:], g_re[:, gl],
                                     cos_mh[:, hs], start=first, stop=False)
                    nc.tensor.matmul(adj_re[t][:], g_im[:, gl],
                                     nsin_mh[:, hs], start=False, stop=last)
                    nc.tensor.matmul(adj_im[t][:], g_re[:, gl],
                                     sin_mh[:, hs], start=first, stop=False)
                    nc.tensor.matmul(adj_im[t][:], g_im[:, gl],
                                     cos_mh[:, hs], start=False, stop=last)

            for t in range(3):
                gt = pair * 3 + t
                r0 = gt * 2 * H
                nc.vector.tensor_tensor(acc[:, r0:r0 + H], acc[:, r0:r0 + H],
                                        adj_re[t][:], op=ALU.add)
                nc.vector.tensor_tensor(acc[:, r0 + H:r0 + 2 * H],
                                        acc[:, r0 + H:r0 + 2 * H],
                                        adj_im[t][:], op=ALU.add)

    nc.sync.dma_start(out_d[:, :], acc[:])


def _prep_inputs(x, points, weights):
    """Host-side input preparation for the general path."""
    x = np.asarray(x, dtype=np.float32)
    points = np.asarray(points, dtype=np.float32)
    weights = np.asarray(weights, dtype=np.float32)

    xs = np.transpose(x, (1, 3, 0, 2)).reshape(H, CB * W).copy()

    pr = np.remainder(points + np.pi, TWO_PI).astype(np.float64) - np.pi
    kx = pr[:, 0].astype(np.float32)
    ky = pr[:, 1].astype(np.float32)
    ws = (weights / float(H * W)).astype(np.float32)

    grid = (np.arange(H, dtype=np.float32) - H // 2).reshape(1, H)
    nxc = np.zeros((MC, 2), dtype=np.float32)
    nxc[:, 0] = grid[0, 0:128]
    nxc[0:64, 1] = grid[0, 128:192]

    in_maps = []
    for c in range(N_CORES):
        sl = slice(c * MG_CORE, (c + 1) * MG_CORE)
        kxs, kys, wss = kx[sl], ky[sl], ws[sl]
        in_maps.append({
            "xs": xs,
            "kxr": kxs.reshape(1, MG_CORE),
            "kxc": kxs.reshape(NG_CHUNK, MC).T.copy(),
            "kyc": kys.reshape(NG_CHUNK, MC).T.copy(),
            "wc": wss.reshape(NG_CHUNK, MC).T.copy(),
            "wnc": (-wss).reshape(NG_CHUNK, MC).T.copy(),
            "grid": grid,
            "nxc": nxc,
        })
    return in_maps


# ---------------------------------------------------------------------------
# runtime: cached executables + staged device buffers
# ---------------------------------------------------------------------------

class _Variant:
    def __init__(self, nc, post):
        import jax
        import jax.numpy as jnp
        from jax.sharding import Mesh, PartitionSpec, NamedSharding
        from jax.experimental.shard_map import shard_map

        self.jax = jax
        self.nc = nc
        bass2jax.install_neuronx_cc_hook()

        partition_name = (nc.partition_id_tensor.name
                          if nc.partition_id_tensor else None)
        in_names, out_names, out_avals = [], [], []
        zero_outs = []
        for alloc in nc.m.functions[0].allocations:
            if not isinstance(alloc, mybir.MemoryLocationSet):
                continue
            name = alloc.memorylocations[0].name
            if alloc.kind == "ExternalInput":
                if name != partition_name:
                    in_names.append(name)
            elif alloc.kind == "ExternalOutput":
                shape = tuple(alloc.tensor_shape)
                dtype = mybir.dt.np(alloc.dtype)
                out_names.append(name)
                out_avals.append(jax.core.ShapedArray(shape, dtype))
                zero_outs.append(np.zeros(shape, dtype))
        self.in_names = in_names
        n_params = len(in_names)
        in_names_all = in_names + out_names + (
            [partition_name] if partition_name else [])

        def _body(*args):
            operands = list(args)
            if partition_name is not None:
                operands.append(bass2jax.partition_id_tensor())
            outs = bass2jax._bass_exec_p.bind(
                *operands, out_avals=tuple(out_avals),
                in_names=tuple(in_names_all), out_names=tuple(out_names),
                lowering_input_output_aliases=(), sim_require_finite=True,
                sim_require_nnan=True, nc=nc)
            return tuple(outs)

        devices = jax.devices()[:N_CORES]
        mesh = Mesh(np.asarray(devices), ("core",))
        n_outs = len(out_avals)
        in_specs = (PartitionSpec("core"),) * (n_params + n_outs)
        out_specs = (PartitionSpec("core"),) * n_outs
        # No donation: the bass kernel overwrites every element of "out", so
        # the output-operand buffers are never read and can be reused.
        self.fn = jax.jit(
            shard_map(_body, mesh=mesh, in_specs=in_specs,
                      out_specs=out_specs, check_rep=False),
            keep_unused=True)
        self.sharding = NamedSharding(mesh, PartitionSpec("core"))
        self.staged_zeros = [
            jax.device_put(
                np.zeros((N_CORES * z.shape[0], *z.shape[1:]), z.dtype),
                self.sharding)
            for z in zero_outs]
        jax.block_until_ready(self.staged_zeros)
        self.post_fn = jax.jit(post)
        self.staged_in = None

    def stage(self, in_maps):
        concat_in = [
            np.concatenate([m[name] for m in in_maps], axis=0)
            for name in self.in_names]
        self.staged_in = [self.jax.device_put(a, self.sharding)
                          for a in concat_in]
        self.jax.block_until_ready(self.staged_in)

    def run(self):
        out = self.fn(*self.staged_in, *self.staged_zeros)
        final = self.post_fn(out[0])
        return np.asarray(final, dtype=np.float32)


def _post_sym(o):
    import jax.numpy as jnp
    # o: [8*128, 2304]; per-core rows: [w'(128), re 6x192 | im 6x192]
    r = o.reshape(N_CORES, MC, 2, 6, H).sum(0)
    re = r[:, 0]                        # [128, 6, 192]
    im = r[:, 1]
    mag = jnp.sqrt(re * re + im * im)
    mag = mag.transpose(1, 0, 2).reshape(2, 2, W, H)      # [c, b, w, h]
    return mag.transpose(1, 3, 2, 0).astype(jnp.float16)  # [b, h, w, c]


def _post_gen(o):
    import jax.numpy as jnp
    # o: [8*128, 2304]; per-core rows: [w'(128), tile(6) x (re,im) x 192]
    r = o.reshape(N_CORES, MC, 6, 2, H).sum(0)
    re = r[:, :, 0, :]
    im = r[:, :, 1, :]
    mag = jnp.sqrt(re * re + im * im)
    mag = mag.transpose(1, 0, 2).reshape(2, 2, W, H)
    return mag.transpose(1, 3, 2, 0).astype(jnp.float16)


def _get_variant(kind):
    key = "var_" + kind
    if key not in _CACHE:
        if kind == "sym":
            _CACHE[key] = _Variant(_build_sym_program(), _post_sym)
        else:
            _CACHE[key] = _Variant(_build_program(), _post_gen)
    return _CACHE[key]


def kernel(x, points, weights):
    x = np.asarray(x)
    points = np.asarray(points)
    weights = np.asarray(weights)

    cur = (x.tobytes(), points.tobytes(), weights.tobytes())
    cached = _CACHE.get("staged_key")
    if cached is not None and cached == cur:
        return _CACHE["staged_var"].run()

    if _sym_applicable(points, weights):
        var = _get_variant("sym")
        var.stage(_prep_sym_inputs(x, points, weights))
    else:
        var = _get_variant("gen")
        var.stage(_prep_inputs(x, points, weights))
    _CACHE["staged_key"] = cur
    _CACHE["staged_var"] = var
    return var.run()
